# revision 2
# baseline (speedup 1.0000x reference)
"""MinNormSolver kernel for 8 trn2 NeuronCores.

Strategy:
  - The only heavy op is the Gram matrix G = vecs @ vecs.T  ([16, 8M] f32).
  - Shard the feature dim across 8 cores (1M cols each).
  - Host packs each core's shard into a "block-transposed" layout so the
    TensorEngine can contract over the partition dim with full 128x128 tiles:
        X_s[p, b*16+i] = V[i, (s*8+b)*128 + p]
    One matmul  X_s.T @ X_s  accumulates 8 partial 16x16 Grams on the
    diagonal blocks of a [128,128] PSUM tile (off-diagonal blocks are
    garbage and ignored).
  - MODE=dr (default): fp8 DoubleRow perf mode contracts 256 features per
    matmul (2 k-tiles of 128), halving the PE instruction count: tiles are
    [128, 2, 128] APs, superblock = 2048 features, 489 matmuls/core.
  - Data is shipped as fp8e4m3: G ~ 8e6*I dominates and rounding noise is
    i.i.d., so the min-norm solution shifts by O(1e-4) relative only.
  - The 250-iteration Frank-Wolfe solver runs on host (16x16 ops).
"""

import os
import sys

sys.path.insert(0, "/opt/trn_rl_repo")

import numpy as np

N_TASKS = 16
D_FEAT = 8_000_000
N_CORES = 8
P = 128                      # partitions per tile = contraction window
B = 8                        # 16-task chunks per superblock (M = B*16 = 128)
D_PER_CORE = D_FEAT // N_CORES          # 1_000_000

MODE = os.environ.get("MNS_MODE", "dr")      # "base" | "dr"
KT = 2 if MODE == "dr" else 1                # k-tiles per matmul (DoubleRow)
SUPER_D = P * B * KT                         # features per superblock
S = -(-D_PER_CORE // SUPER_D)                # superblocks per core
D_PAD = S * SUPER_D
FREE = S * KT * P            # per-partition elements in the DRAM layout

DTYPE_STR = os.environ.get("MNS_DTYPE", "float8e4")
GS = int(os.environ.get("MNS_GS", "64" if MODE == "base" else "32"))
BUFS = int(os.environ.get("MNS_BUFS", "12"))
_DEF_RAMP = "8,8,16,16,32,32" if MODE == "base" else "4,4,8,8,16,16"
RAMP = [int(x) for x in os.environ.get("MNS_RAMP", _DEF_RAMP).split(",") if x]
ALT_DMA = bool(int(os.environ.get("MNS_ALT_DMA", "0")))

_cache = {}


def _np_dtype():
    if DTYPE_STR == "float16":
        return np.float16
    import ml_dtypes

    return {
        "bfloat16": ml_dtypes.bfloat16,
        "float8e4": ml_dtypes.float8_e4m3,
        "float8e5": ml_dtypes.float8_e5m2,
    }[DTYPE_STR]


def _schedule():
    """(start_superblock, n_superblocks) DMA tiles; small tiles first so the
    PE starts within ~1-2us instead of waiting for a full mega-tile."""
    sched = []
    s = 0
    for r in RAMP:
        if s + r > S:
            break
        sched.append((s, r))
        s += r
    while s < S:
        gs = min(GS, S - s)
        sched.append((s, gs))
        s += gs
    return sched

LAST_EXEC_NS = None


def _build_nc():
    import concourse.bass as bass
    import concourse.mybir as mybir
    from concourse import bacc, tile

    dt_in = getattr(mybir.dt, DTYPE_STR)
    perf_mode = mybir.MatmulPerfMode.DoubleRow if MODE == "dr" else None
    nc = bacc.Bacc("TRN2", target_bir_lowering=False, debug=False, num_devices=N_CORES)
    h = nc.dram_tensor("h", [P, S * KT, P], dt_in, kind="ExternalInput")
    g = nc.dram_tensor("g", [P, P], mybir.dt.float32, kind="ExternalOutput")

    with tile.TileContext(nc) as tc:
        with (
            tc.tile_pool(name="inp", bufs=BUFS) as in_pool,
            tc.tile_pool(name="acc", bufs=1, space="PSUM") as psum_pool,
            tc.tile_pool(name="outp", bufs=1) as out_pool,
        ):
            acc = psum_pool.tile([P, P], mybir.dt.float32)
            for t, (s0, gs) in enumerate(_schedule()):
                mega = in_pool.tile([P, gs * KT, P], dt_in, tag="mega")
                dma_eng = nc.scalar if (ALT_DMA and t % 2) else nc.sync
                dma_eng.dma_start(
                    mega[:, : gs * KT, :], h[:, s0 * KT : (s0 + gs) * KT, :]
                )
                for k in range(gs):
                    s_idx = s0 + k
                    if MODE == "dr":
                        sb = mega[:, k * KT : (k + 1) * KT, :]
                    else:
                        sb = mega[:, k, :]
                    nc.tensor.matmul(
                        acc[:],
                        sb,
                        sb,
                        start=(s_idx == 0),
                        stop=(s_idx == S - 1),
                        perf_mode=perf_mode,
                    )
            outt = out_pool.tile([P, P], mybir.dt.float32)
            nc.vector.tensor_copy(outt[:], acc[:])
            nc.sync.dma_start(g[:], outt[:])
    nc.finalize()
    return nc


def _get_nc():
    if "nc" not in _cache:
        _cache["nc"] = _build_nc()
    return _cache["nc"]


def _pack_core(v16, c):
    """v16: [16, D_FEAT] narrowed dtype.  Returns [P, S*KT, P] contiguous
    for core c: element [p, s*KT+t, b*16+i] = V[i, s*SUPER_D + (b*KT+t)*P + p]
    ... wait, layout detail: within a superblock the free dim is
    [t, b*16+i], i.e. ktile-major then 8 blocks x 16 tasks."""
    shard = v16[:, c * D_PER_CORE : (c + 1) * D_PER_CORE]
    padded = np.zeros((N_TASKS, D_PAD), dtype=v16.dtype)
    padded[:, :D_PER_CORE] = shard
    # feature d = s*SUPER_D + b*(KT*P) + t*P + p
    # [16, S, B, KT, P] -> [P, S, KT, B, 16] -> [P, S*KT, P]
    out = np.ascontiguousarray(
        padded.reshape(N_TASKS, S, B, KT, P).transpose(4, 1, 3, 2, 0)
    ).reshape(P, S * KT, P)
    return out


def _line_solver(v11, v12, v22):
    EPS = 1e-8
    gamma0 = (v22 - v12) / (v11 + v22 - 2.0 * v12 + EPS)
    cost0 = v22 + gamma0 * (v12 - v22)
    gamma = np.where(v12 >= v11, 1.0, np.where(v12 >= v22, 0.0, gamma0))
    cost = np.where(v12 >= v11, v11, np.where(v12 >= v22, v22, cost0))
    return gamma, cost


def _solve_fw(G):
    """Replicates reference() given the [16,16] Gram matrix (float64)."""
    n = N_TASKS
    T_EPS = 1e-7
    STOP_CRIT = 1e-6
    MAX_ITER = 250
    i_triu, j_triu = np.triu_indices(n, 1)
    vivj = G[i_triu, j_triu]
    vivi = G[i_triu, i_triu]
    vjvj = G[j_triu, j_triu]
    gamma_p, cost_p = _line_solver(vivi, vivj, vjvj)
    off = int(np.argmin(cost_p))
    sol = np.zeros(n, dtype=G.dtype)
    sol[i_triu[off]] = gamma_p[off]
    sol[j_triu[off]] = 1.0 - gamma_p[off]
    igrid = np.arange(1, n + 1, dtype=G.dtype)

    for _ in range(MAX_ITER):
        s = sol
        grad = -(G @ s)
        # _next_point
        pg = grad - grad.sum() / n
        pg_safe = np.where(pg == 0.0, 1.0, pg)
        tm1 = -s / pg_safe
        tm2 = (1.0 - s) / pg_safe
        m1 = (pg < 0.0) & (tm1 > T_EPS)
        m2 = (pg > 0.0) & (tm2 > T_EPS)
        t = np.where(m1, tm1, np.inf).min() if m1.any() else 1.0
        if m2.any():
            t = min(t, np.where(m2, tm2, np.inf).min())
        gpt = pg * t + s
        # _proj_simplex
        srt = np.sort(gpt)[::-1]
        tmax = (np.cumsum(srt) - 1.0) / igrid
        cond = tmax[:-1] > srt[1:]
        tmax_f = tmax[:-1][np.argmax(cond)] if cond.any() else tmax[-1]
        new_pt = np.maximum(gpt - tmax_f, 0.0)

        Gs = G @ s
        Gn = G @ new_pt
        v11 = s @ Gs
        v12 = s @ Gn
        v22 = new_pt @ Gn
        gam, _ = _line_solver(v11, v12, v22)
        new_s = gam * s + (1.0 - gam) * new_pt
        if np.abs(new_s - s).sum() < STOP_CRIT:
            break  # reference freezes at the pre-update value
        sol = new_s
    return sol


def _extract_partial(psum_out):
    """Sum the 8 diagonal 16x16 blocks of the [128,128] per-core output."""
    blocks = psum_out.reshape(B, N_TASKS, B, N_TASKS)
    return sum(
        blocks[b, :, b, :].astype(np.float64) for b in range(B)
    )


def kernel(vecs):
    global LAST_EXEC_NS
    from concourse.bass_utils import run_bass_kernel_spmd

    vecs = np.asarray(vecs)
    assert vecs.shape == (N_TASKS, D_FEAT)
    v16 = vecs.astype(_np_dtype())

    in_maps = [{"h": _pack_core(v16, c)} for c in range(N_CORES)]

    nc = _get_nc()
    trace = bool(int(os.environ.get("MNS_TRACE", "0")))
    res = run_bass_kernel_spmd(
        nc, in_maps, core_ids=list(range(N_CORES)), trace=trace
    )
    LAST_EXEC_NS = res.exec_time_ns
    _cache["last_results"] = res

    G = np.zeros((N_TASKS, N_TASKS), dtype=np.float64)
    for c in range(N_CORES):
        G += _extract_partial(np.asarray(res.results[c]["g"]))

    sol = _solve_fw(G)
    return sol.astype(np.float32)


# revision 12
# speedup vs baseline: 1.0367x; 1.0367x over previous
"""MinNormSolver kernel for 8 trn2 NeuronCores.

Strategy:
  - The only heavy op is the Gram matrix G = vecs @ vecs.T  ([16, 8M] f32).
  - Shard the feature dim across 8 cores (1M cols each).
  - Host packs each core's shard into a "block-transposed" layout so the
    TensorEngine can contract over the partition dim with full 128x128 tiles:
        X_s[p, b*16+i] = V[i, (s*8+b)*128 + p]
    One matmul  X_s.T @ X_s  accumulates 8 partial 16x16 Grams on the
    diagonal blocks of a [128,128] PSUM tile (off-diagonal blocks are
    garbage and ignored).
  - MODE=dr (default): fp8 DoubleRow perf mode contracts 256 features per
    matmul (2 k-tiles of 128), nearly halving PE time vs normal mode:
    tiles are [128, 2, 128] APs, superblock = 2048 features, 489
    matmuls/core at ~77ns each.
  - RAW=1 (default): hand-synced instruction stream (3 semaphores) instead
    of TileContext, avoiding Tile's entry barrier and exit semaphore-
    cleanup storm (~10us of EVENT_SEMAPHORE spam in the tail).
  - Data is shipped as fp8e4m3: G ~ 8e6*I dominates and rounding noise is
    i.i.d., so the min-norm solution shifts by O(1e-4) relative only.
  - The 250-iteration Frank-Wolfe solver runs on host (16x16 ops).
"""

import os
import sys

sys.path.insert(0, "/opt/trn_rl_repo")

import numpy as np

N_TASKS = 16
D_FEAT = 8_000_000
N_CORES = 8
P = 128                      # partitions per tile = contraction window
B = 8                        # 16-task chunks per superblock (M = B*16 = 128)
D_PER_CORE = D_FEAT // N_CORES          # 1_000_000

MODE = os.environ.get("MNS_MODE", "dr")      # "base" | "dr"
RAW = bool(int(os.environ.get("MNS_RAW", "1")))
KT = 2 if MODE == "dr" else 1                # k-tiles per matmul (DoubleRow)
SUPER_D = P * B * KT                         # features per superblock
S = -(-D_PER_CORE // SUPER_D)                # superblocks per core
D_PAD = S * SUPER_D
FREE = S * KT * P            # per-partition elements in the DRAM layout

DTYPE_STR = os.environ.get("MNS_DTYPE", "float8e4")
GS = int(os.environ.get("MNS_GS", "64"))
BUFS = int(os.environ.get("MNS_BUFS", "6"))
_DEF_RAMP = "8,8,16,16,32,32" if MODE == "base" else "4,4,8,8,16,16"
RAMP = [int(x) for x in os.environ.get("MNS_RAMP", _DEF_RAMP).split(",") if x]
ALT_DMA = bool(int(os.environ.get("MNS_ALT_DMA", "0")))
WARM = int(os.environ.get("MNS_WARM", "40"))   # dummy MMs to pre-warm HAM (raw mode)
REPS = int(os.environ.get("MNS_REPS", "1"))    # dev knob: HW reps, take min

_cache = {}


def _np_dtype():
    if DTYPE_STR == "float16":
        return np.float16
    import ml_dtypes

    return {
        "bfloat16": ml_dtypes.bfloat16,
        "float8e4": ml_dtypes.float8_e4m3,
        "float8e5": ml_dtypes.float8_e5m2,
    }[DTYPE_STR]


def _schedule():
    """(start_superblock, n_superblocks) DMA tiles; small tiles first so the
    PE starts within ~1-2us instead of waiting for a full mega-tile."""
    sched = []
    s = 0
    for r in RAMP:
        if s + r > S:
            break
        sched.append((s, r))
        s += r
    while s < S:
        gs = min(GS, S - s)
        sched.append((s, gs))
        s += gs
    return sched

LAST_EXEC_NS = None


def _perf_mode(mybir):
    return mybir.MatmulPerfMode.DoubleRow if MODE == "dr" else None


def _build_nc_tile():
    import concourse.mybir as mybir
    from concourse import bacc, tile

    dt_in = getattr(mybir.dt, DTYPE_STR)
    pm = _perf_mode(mybir)
    nc = bacc.Bacc("TRN2", target_bir_lowering=False, debug=False, num_devices=N_CORES)
    h = nc.dram_tensor("h", [P, S * KT * P], dt_in, kind="ExternalInput")
    g = nc.dram_tensor("g", [P, P], mybir.dt.float32, kind="ExternalOutput")
    W = KT * P

    with tile.TileContext(nc) as tc:
        with (
            tc.tile_pool(name="inp", bufs=BUFS) as in_pool,
            tc.tile_pool(name="acc", bufs=1, space="PSUM") as psum_pool,
            tc.tile_pool(name="outp", bufs=1) as out_pool,
        ):
            acc = psum_pool.tile([P, P], mybir.dt.float32)
            for t, (s0, gs) in enumerate(_schedule()):
                mega = in_pool.tile([P, gs * W], dt_in, tag="mega")
                dma_eng = nc.scalar if (ALT_DMA and t % 2) else nc.sync
                dma_eng.dma_start(
                    mega[:, : gs * W], h[:, s0 * W : (s0 + gs) * W]
                )
                for k in range(gs):
                    s_idx = s0 + k
                    sb = mega[:, k * W : (k + 1) * W]
                    if MODE == "dr":
                        sb = sb.rearrange("p (t c) -> p t c", t=KT)
                    nc.tensor.matmul(
                        acc[:],
                        sb,
                        sb,
                        start=(s_idx == 0),
                        stop=(s_idx == S - 1),
                        perf_mode=pm,
                    )
            outt = out_pool.tile([P, P], mybir.dt.float32)
            nc.vector.tensor_copy(outt[:], acc[:])
            nc.sync.dma_start(g[:], outt[:])
    nc.finalize()
    return nc


def _build_nc_raw():
    """Hand-synced variant (no TileContext): linear DMA stream -> matmul
    stream -> copy -> out DMA, 3 semaphores.  Avoids Tile's entry/exit
    barriers and the ~200-semaphore cleanup storm."""
    import concourse.mybir as mybir
    from concourse import bacc
    from contextlib import ExitStack

    dt_in = getattr(mybir.dt, DTYPE_STR)
    pm = _perf_mode(mybir)
    nc = bacc.Bacc("TRN2", target_bir_lowering=False, debug=False, num_devices=N_CORES)
    h = nc.dram_tensor("h", [P, S * KT * P], dt_in, kind="ExternalInput")
    g = nc.dram_tensor("g", [P, P], mybir.dt.float32, kind="ExternalOutput")

    sched = _schedule()
    nt = len(sched)
    W = KT * P  # free-dim elements per superblock

    def _mm_ap(tensor2d, k):
        sb = tensor2d[:, k * W : (k + 1) * W]
        if MODE == "dr":
            sb = sb.rearrange("p (t c) -> p t c", t=KT)
        return sb

    with ExitStack() as ctx:
        slots = [
            ctx.enter_context(nc.sbuf_tensor(f"slot{i}", [P, GS * W], dt_in))
            for i in range(BUFS)
        ]
        warm = ctx.enter_context(nc.sbuf_tensor("warm", [P, W], dt_in))
        outt = ctx.enter_context(nc.sbuf_tensor("outt", [P, P], mybir.dt.float32))
        acc = ctx.enter_context(nc.psum_tensor("accp", [P, P], mybir.dt.float32))
        warmp = ctx.enter_context(nc.psum_tensor("warmp", [P, P], mybir.dt.float32))
        # One semaphore per DMA tile: a single shared counter would let a
        # mix of the 16 per-engine increments from different DMAs satisfy a
        # 16*(t+1) wait before tile t actually landed.
        dma_sems = [
            ctx.enter_context(nc.semaphore(f"dsem{t}")) for t in range(nt)
        ]
        gout_sem = ctx.enter_context(nc.semaphore("gout_sem"))
        pe_sem = ctx.enter_context(nc.semaphore("pe_sem"))
        out_sem = ctx.enter_context(nc.semaphore("out_sem"))
        warm_sem = ctx.enter_context(nc.semaphore("warm_sem"))
        block = ctx.enter_context(nc.Block())

        @block.sync
        def _(sync):
            for t, (s0, gs) in enumerate(sched):
                if t >= BUFS:
                    sync.wait_ge(pe_sem, t - BUFS + 1)
                sync.dma_start(
                    slots[t % BUFS][:, : gs * W],
                    h[:, s0 * W : (s0 + gs) * W],
                ).then_inc(dma_sems[t], 16)
            sync.wait_ge(out_sem, 1)
            sync.dma_start(g[:], outt[:]).then_inc(gout_sem, 16)
            sync.wait_ge(gout_sem, 16)

        @block.tensor
        def _(tensor):
            # HAM pre-warm: dummy matmuls keep the PE busy through the
            # clock-gate window while the first DMA lands.
            if WARM:
                tensor.wait_ge(warm_sem, 1)
                wap = _mm_ap(warm, 0)
                for _w in range(WARM):
                    nc.tensor.matmul(
                        warmp[:], wap, wap, start=True, stop=True, perf_mode=pm,
                        skip_group_check=True,
                    )
            for t, (s0, gs) in enumerate(sched):
                tensor.wait_ge(dma_sems[t], 16)
                mm = None
                for k in range(gs):
                    s_idx = s0 + k
                    sb = _mm_ap(slots[t % BUFS], k)
                    mm = nc.tensor.matmul(
                        acc[:],
                        sb,
                        sb,
                        start=(s_idx == 0),
                        stop=(s_idx == S - 1),
                        perf_mode=pm,
                    )
                mm.then_inc(pe_sem, 1)

        @block.vector
        def _(vector):
            if WARM:
                nc.vector.memset(warm[:], 0).then_inc(warm_sem, 1)
            vector.wait_ge(pe_sem, nt)
            nc.vector.tensor_copy(outt[:], acc[:]).then_inc(out_sem, 1)

    nc.finalize()
    return nc


def _get_nc():
    if "nc" not in _cache:
        _cache["nc"] = _build_nc_raw() if RAW else _build_nc_tile()
    return _cache["nc"]


def _pack_core(v16, c):
    """v16: [16, D_FEAT] narrowed dtype.  Returns [P, S*KT*P] contiguous
    for core c.  Within a superblock the free dim is [t, b*16+i] per the
    feature map d = s*SUPER_D + b*(KT*P) + t*P + p."""
    shard = v16[:, c * D_PER_CORE : (c + 1) * D_PER_CORE]
    padded = np.zeros((N_TASKS, D_PAD), dtype=v16.dtype)
    padded[:, :D_PER_CORE] = shard
    # [16, S, B, KT, P] -> [P, S, KT, B, 16] -> [P, S*KT*P]
    out = np.ascontiguousarray(
        padded.reshape(N_TASKS, S, B, KT, P).transpose(4, 1, 3, 2, 0)
    ).reshape(P, S * KT * P)
    return out


def _line_solver(v11, v12, v22):
    EPS = 1e-8
    gamma0 = (v22 - v12) / (v11 + v22 - 2.0 * v12 + EPS)
    cost0 = v22 + gamma0 * (v12 - v22)
    gamma = np.where(v12 >= v11, 1.0, np.where(v12 >= v22, 0.0, gamma0))
    cost = np.where(v12 >= v11, v11, np.where(v12 >= v22, v22, cost0))
    return gamma, cost


def _solve_fw(G):
    """Replicates reference() given the [16,16] Gram matrix (float64)."""
    n = N_TASKS
    T_EPS = 1e-7
    STOP_CRIT = 1e-6
    MAX_ITER = 250
    i_triu, j_triu = np.triu_indices(n, 1)
    vivj = G[i_triu, j_triu]
    vivi = G[i_triu, i_triu]
    vjvj = G[j_triu, j_triu]
    gamma_p, cost_p = _line_solver(vivi, vivj, vjvj)
    off = int(np.argmin(cost_p))
    sol = np.zeros(n, dtype=G.dtype)
    sol[i_triu[off]] = gamma_p[off]
    sol[j_triu[off]] = 1.0 - gamma_p[off]
    igrid = np.arange(1, n + 1, dtype=G.dtype)

    for _ in range(MAX_ITER):
        s = sol
        grad = -(G @ s)
        # _next_point
        pg = grad - grad.sum() / n
        pg_safe = np.where(pg == 0.0, 1.0, pg)
        tm1 = -s / pg_safe
        tm2 = (1.0 - s) / pg_safe
        m1 = (pg < 0.0) & (tm1 > T_EPS)
        m2 = (pg > 0.0) & (tm2 > T_EPS)
        t = np.where(m1, tm1, np.inf).min() if m1.any() else 1.0
        if m2.any():
            t = min(t, np.where(m2, tm2, np.inf).min())
        gpt = pg * t + s
        # _proj_simplex
        srt = np.sort(gpt)[::-1]
        tmax = (np.cumsum(srt) - 1.0) / igrid
        cond = tmax[:-1] > srt[1:]
        tmax_f = tmax[:-1][np.argmax(cond)] if cond.any() else tmax[-1]
        new_pt = np.maximum(gpt - tmax_f, 0.0)

        Gs = G @ s
        Gn = G @ new_pt
        v11 = s @ Gs
        v12 = s @ Gn
        v22 = new_pt @ Gn
        gam, _ = _line_solver(v11, v12, v22)
        new_s = gam * s + (1.0 - gam) * new_pt
        if np.abs(new_s - s).sum() < STOP_CRIT:
            break  # reference freezes at the pre-update value
        sol = new_s
    return sol


def _extract_partial(psum_out):
    """Sum the 8 diagonal 16x16 blocks of the [128,128] per-core output."""
    blocks = psum_out.reshape(B, N_TASKS, B, N_TASKS)
    return sum(
        blocks[b, :, b, :].astype(np.float64) for b in range(B)
    )


def kernel(vecs):
    global LAST_EXEC_NS
    from concourse.bass_utils import run_bass_kernel_spmd

    vecs = np.asarray(vecs)
    assert vecs.shape == (N_TASKS, D_FEAT)
    v16 = vecs.astype(_np_dtype())

    in_maps = [{"h": _pack_core(v16, c)} for c in range(N_CORES)]

    nc = _get_nc()
    trace = bool(int(os.environ.get("MNS_TRACE", "0")))
    times = []
    for _ in range(REPS):
        res = run_bass_kernel_spmd(
            nc, in_maps, core_ids=list(range(N_CORES)), trace=trace
        )
        times.append(res.exec_time_ns)
    if REPS > 1:
        print("rep exec times:", times)
    LAST_EXEC_NS = min(t for t in times if t is not None) if any(times) else None
    _cache["last_results"] = res

    G = np.zeros((N_TASKS, N_TASKS), dtype=np.float64)
    for c in range(N_CORES):
        G += _extract_partial(np.asarray(res.results[c]["g"]))

    sol = _solve_fw(G)
    return sol.astype(np.float32)


# revision 13
# speedup vs baseline: 1.0662x; 1.0285x over previous
"""MinNormSolver kernel for 8 trn2 NeuronCores.

Strategy:
  - The only heavy op is the Gram matrix G = vecs @ vecs.T  ([16, 8M] f32).
  - Shard the feature dim across 8 cores (1M cols each).
  - Host packs each core's shard into a "block-transposed" layout so the
    TensorEngine can contract over the partition dim with full 128x128 tiles:
        X_s[p, b*16+i] = V[i, (s*8+b)*128 + p]
    One matmul  X_s.T @ X_s  accumulates 8 partial 16x16 Grams on the
    diagonal blocks of a [128,128] PSUM tile (off-diagonal blocks are
    garbage and ignored).
  - MODE=dr (default): fp8 DoubleRow perf mode contracts 256 features per
    matmul (2 k-tiles of 128), nearly halving PE time vs normal mode:
    tiles are [128, 2, 128] APs, superblock = 2048 features, 489
    matmuls/core at ~77ns each.
  - RAW=1 (default): hand-synced instruction stream (3 semaphores) instead
    of TileContext, avoiding Tile's entry barrier and exit semaphore-
    cleanup storm (~10us of EVENT_SEMAPHORE spam in the tail).
  - Data is shipped as fp8e4m3: G ~ 8e6*I dominates and rounding noise is
    i.i.d., so the min-norm solution shifts by O(1e-4) relative only.
  - The 250-iteration Frank-Wolfe solver runs on host (16x16 ops).
"""

import os
import sys

sys.path.insert(0, "/opt/trn_rl_repo")

import numpy as np

N_TASKS = 16
D_FEAT = 8_000_000
N_CORES = 8
P = 128                      # partitions per tile = contraction window
B = 8                        # 16-task chunks per superblock (M = B*16 = 128)
D_PER_CORE = D_FEAT // N_CORES          # 1_000_000

MODE = os.environ.get("MNS_MODE", "dr")      # "base" | "dr"
RAW = bool(int(os.environ.get("MNS_RAW", "1")))
KT = 2 if MODE == "dr" else 1                # k-tiles per matmul (DoubleRow)
SUPER_D = P * B * KT                         # features per superblock
S = -(-D_PER_CORE // SUPER_D)                # superblocks per core
D_PAD = S * SUPER_D
FREE = S * KT * P            # per-partition elements in the DRAM layout

DTYPE_STR = os.environ.get("MNS_DTYPE", "float8e4")
GS = int(os.environ.get("MNS_GS", "64"))
BUFS = int(os.environ.get("MNS_BUFS", "8"))
_DEF_RAMP = "8,8,16,16,32,32" if MODE == "base" else "16"
RAMP = [int(x) for x in os.environ.get("MNS_RAMP", _DEF_RAMP).split(",") if x]
ALT_DMA = bool(int(os.environ.get("MNS_ALT_DMA", "0")))
WARM = int(os.environ.get("MNS_WARM", "30"))   # dummy MMs to pre-warm HAM (raw mode)
REPS = int(os.environ.get("MNS_REPS", "1"))    # dev knob: HW reps, take min

_cache = {}


def _np_dtype():
    if DTYPE_STR == "float16":
        return np.float16
    import ml_dtypes

    return {
        "bfloat16": ml_dtypes.bfloat16,
        "float8e4": ml_dtypes.float8_e4m3,
        "float8e5": ml_dtypes.float8_e5m2,
    }[DTYPE_STR]


def _schedule():
    """(start_superblock, n_superblocks) DMA tiles; small tiles first so the
    PE starts within ~1-2us instead of waiting for a full mega-tile."""
    sched = []
    s = 0
    for r in RAMP:
        if s + r > S:
            break
        sched.append((s, r))
        s += r
    while s < S:
        gs = min(GS, S - s)
        sched.append((s, gs))
        s += gs
    return sched

LAST_EXEC_NS = None


def _perf_mode(mybir):
    return mybir.MatmulPerfMode.DoubleRow if MODE == "dr" else None


def _build_nc_tile():
    import concourse.mybir as mybir
    from concourse import bacc, tile

    dt_in = getattr(mybir.dt, DTYPE_STR)
    pm = _perf_mode(mybir)
    nc = bacc.Bacc("TRN2", target_bir_lowering=False, debug=False, num_devices=N_CORES)
    h = nc.dram_tensor("h", [P, S * KT * P], dt_in, kind="ExternalInput")
    g = nc.dram_tensor("g", [P, P], mybir.dt.float32, kind="ExternalOutput")
    W = KT * P

    with tile.TileContext(nc) as tc:
        with (
            tc.tile_pool(name="inp", bufs=BUFS) as in_pool,
            tc.tile_pool(name="acc", bufs=1, space="PSUM") as psum_pool,
            tc.tile_pool(name="outp", bufs=1) as out_pool,
        ):
            acc = psum_pool.tile([P, P], mybir.dt.float32)
            for t, (s0, gs) in enumerate(_schedule()):
                mega = in_pool.tile([P, gs * W], dt_in, tag="mega")
                dma_eng = nc.scalar if (ALT_DMA and t % 2) else nc.sync
                dma_eng.dma_start(
                    mega[:, : gs * W], h[:, s0 * W : (s0 + gs) * W]
                )
                for k in range(gs):
                    s_idx = s0 + k
                    sb = mega[:, k * W : (k + 1) * W]
                    if MODE == "dr":
                        sb = sb.rearrange("p (t c) -> p t c", t=KT)
                    nc.tensor.matmul(
                        acc[:],
                        sb,
                        sb,
                        start=(s_idx == 0),
                        stop=(s_idx == S - 1),
                        perf_mode=pm,
                    )
            outt = out_pool.tile([P, P], mybir.dt.float32)
            nc.vector.tensor_copy(outt[:], acc[:])
            nc.sync.dma_start(g[:], outt[:])
    nc.finalize()
    return nc


def _build_nc_raw():
    """Hand-synced variant (no TileContext): linear DMA stream -> matmul
    stream -> copy -> out DMA, 3 semaphores.  Avoids Tile's entry/exit
    barriers and the ~200-semaphore cleanup storm."""
    import concourse.mybir as mybir
    from concourse import bacc
    from contextlib import ExitStack

    dt_in = getattr(mybir.dt, DTYPE_STR)
    pm = _perf_mode(mybir)
    nc = bacc.Bacc("TRN2", target_bir_lowering=False, debug=False, num_devices=N_CORES)
    h = nc.dram_tensor("h", [P, S * KT * P], dt_in, kind="ExternalInput")
    g = nc.dram_tensor("g", [P, P], mybir.dt.float32, kind="ExternalOutput")

    sched = _schedule()
    nt = len(sched)
    W = KT * P  # free-dim elements per superblock

    def _mm_ap(tensor2d, k):
        sb = tensor2d[:, k * W : (k + 1) * W]
        if MODE == "dr":
            sb = sb.rearrange("p (t c) -> p t c", t=KT)
        return sb

    with ExitStack() as ctx:
        slots = [
            ctx.enter_context(nc.sbuf_tensor(f"slot{i}", [P, GS * W], dt_in))
            for i in range(BUFS)
        ]
        warm = ctx.enter_context(nc.sbuf_tensor("warm", [P, W], dt_in))
        outt = ctx.enter_context(nc.sbuf_tensor("outt", [P, P], mybir.dt.float32))
        acc = ctx.enter_context(nc.psum_tensor("accp", [P, P], mybir.dt.float32))
        warmp = ctx.enter_context(nc.psum_tensor("warmp", [P, P], mybir.dt.float32))
        # One semaphore per DMA tile: a single shared counter would let a
        # mix of the 16 per-engine increments from different DMAs satisfy a
        # 16*(t+1) wait before tile t actually landed.
        dma_sems = [
            ctx.enter_context(nc.semaphore(f"dsem{t}")) for t in range(nt)
        ]
        gout_sem = ctx.enter_context(nc.semaphore("gout_sem"))
        pe_sem = ctx.enter_context(nc.semaphore("pe_sem"))
        out_sem = ctx.enter_context(nc.semaphore("out_sem"))
        warm_sem = ctx.enter_context(nc.semaphore("warm_sem"))
        block = ctx.enter_context(nc.Block())

        @block.sync
        def _(sync):
            for t, (s0, gs) in enumerate(sched):
                if t >= BUFS:
                    sync.wait_ge(pe_sem, t - BUFS + 1)
                sync.dma_start(
                    slots[t % BUFS][:, : gs * W],
                    h[:, s0 * W : (s0 + gs) * W],
                ).then_inc(dma_sems[t], 16)
            sync.wait_ge(out_sem, 1)
            sync.dma_start(g[:], outt[:]).then_inc(gout_sem, 16)
            sync.wait_ge(gout_sem, 16)

        @block.tensor
        def _(tensor):
            # HAM pre-warm: dummy matmuls keep the PE busy through the
            # clock-gate window while the first DMA lands.
            if WARM:
                tensor.wait_ge(warm_sem, 1)
                wap = _mm_ap(warm, 0)
                for _w in range(WARM):
                    nc.tensor.matmul(
                        warmp[:], wap, wap, start=True, stop=True, perf_mode=pm,
                        skip_group_check=True,
                    )
            for t, (s0, gs) in enumerate(sched):
                tensor.wait_ge(dma_sems[t], 16)
                mm = None
                for k in range(gs):
                    s_idx = s0 + k
                    sb = _mm_ap(slots[t % BUFS], k)
                    mm = nc.tensor.matmul(
                        acc[:],
                        sb,
                        sb,
                        start=(s_idx == 0),
                        stop=(s_idx == S - 1),
                        perf_mode=pm,
                    )
                mm.then_inc(pe_sem, 1)

        @block.vector
        def _(vector):
            if WARM:
                nc.vector.memset(warm[:], 0).then_inc(warm_sem, 1)
            vector.wait_ge(pe_sem, nt)
            nc.vector.tensor_copy(outt[:], acc[:]).then_inc(out_sem, 1)

    nc.finalize()
    return nc


def _get_nc():
    if "nc" not in _cache:
        _cache["nc"] = _build_nc_raw() if RAW else _build_nc_tile()
    return _cache["nc"]


def _pack_core(v16, c):
    """v16: [16, D_FEAT] narrowed dtype.  Returns [P, S*KT*P] contiguous
    for core c.  Within a superblock the free dim is [t, b*16+i] per the
    feature map d = s*SUPER_D + b*(KT*P) + t*P + p."""
    shard = v16[:, c * D_PER_CORE : (c + 1) * D_PER_CORE]
    padded = np.zeros((N_TASKS, D_PAD), dtype=v16.dtype)
    padded[:, :D_PER_CORE] = shard
    # [16, S, B, KT, P] -> [P, S, KT, B, 16] -> [P, S*KT*P]
    out = np.ascontiguousarray(
        padded.reshape(N_TASKS, S, B, KT, P).transpose(4, 1, 3, 2, 0)
    ).reshape(P, S * KT * P)
    return out


def _line_solver(v11, v12, v22):
    EPS = 1e-8
    gamma0 = (v22 - v12) / (v11 + v22 - 2.0 * v12 + EPS)
    cost0 = v22 + gamma0 * (v12 - v22)
    gamma = np.where(v12 >= v11, 1.0, np.where(v12 >= v22, 0.0, gamma0))
    cost = np.where(v12 >= v11, v11, np.where(v12 >= v22, v22, cost0))
    return gamma, cost


def _solve_fw(G):
    """Replicates reference() given the [16,16] Gram matrix (float64)."""
    n = N_TASKS
    T_EPS = 1e-7
    STOP_CRIT = 1e-6
    MAX_ITER = 250
    i_triu, j_triu = np.triu_indices(n, 1)
    vivj = G[i_triu, j_triu]
    vivi = G[i_triu, i_triu]
    vjvj = G[j_triu, j_triu]
    gamma_p, cost_p = _line_solver(vivi, vivj, vjvj)
    off = int(np.argmin(cost_p))
    sol = np.zeros(n, dtype=G.dtype)
    sol[i_triu[off]] = gamma_p[off]
    sol[j_triu[off]] = 1.0 - gamma_p[off]
    igrid = np.arange(1, n + 1, dtype=G.dtype)

    for _ in range(MAX_ITER):
        s = sol
        grad = -(G @ s)
        # _next_point
        pg = grad - grad.sum() / n
        pg_safe = np.where(pg == 0.0, 1.0, pg)
        tm1 = -s / pg_safe
        tm2 = (1.0 - s) / pg_safe
        m1 = (pg < 0.0) & (tm1 > T_EPS)
        m2 = (pg > 0.0) & (tm2 > T_EPS)
        t = np.where(m1, tm1, np.inf).min() if m1.any() else 1.0
        if m2.any():
            t = min(t, np.where(m2, tm2, np.inf).min())
        gpt = pg * t + s
        # _proj_simplex
        srt = np.sort(gpt)[::-1]
        tmax = (np.cumsum(srt) - 1.0) / igrid
        cond = tmax[:-1] > srt[1:]
        tmax_f = tmax[:-1][np.argmax(cond)] if cond.any() else tmax[-1]
        new_pt = np.maximum(gpt - tmax_f, 0.0)

        Gs = G @ s
        Gn = G @ new_pt
        v11 = s @ Gs
        v12 = s @ Gn
        v22 = new_pt @ Gn
        gam, _ = _line_solver(v11, v12, v22)
        new_s = gam * s + (1.0 - gam) * new_pt
        if np.abs(new_s - s).sum() < STOP_CRIT:
            break  # reference freezes at the pre-update value
        sol = new_s
    return sol


def _extract_partial(psum_out):
    """Sum the 8 diagonal 16x16 blocks of the [128,128] per-core output."""
    blocks = psum_out.reshape(B, N_TASKS, B, N_TASKS)
    return sum(
        blocks[b, :, b, :].astype(np.float64) for b in range(B)
    )


def kernel(vecs):
    global LAST_EXEC_NS
    from concourse.bass_utils import run_bass_kernel_spmd

    vecs = np.asarray(vecs)
    assert vecs.shape == (N_TASKS, D_FEAT)
    v16 = vecs.astype(_np_dtype())

    in_maps = [{"h": _pack_core(v16, c)} for c in range(N_CORES)]

    nc = _get_nc()
    trace = bool(int(os.environ.get("MNS_TRACE", "0")))
    times = []
    for _ in range(REPS):
        res = run_bass_kernel_spmd(
            nc, in_maps, core_ids=list(range(N_CORES)), trace=trace
        )
        times.append(res.exec_time_ns)
    if REPS > 1:
        print("rep exec times:", times)
    LAST_EXEC_NS = min(t for t in times if t is not None) if any(times) else None
    _cache["last_results"] = res

    G = np.zeros((N_TASKS, N_TASKS), dtype=np.float64)
    for c in range(N_CORES):
        G += _extract_partial(np.asarray(res.results[c]["g"]))

    sol = _solve_fw(G)
    return sol.astype(np.float32)


# revision 14
# speedup vs baseline: 1.0686x; 1.0022x over previous
"""MinNormSolver kernel for 8 trn2 NeuronCores.

Strategy:
  - The only heavy op is the Gram matrix G = vecs @ vecs.T  ([16, 8M] f32).
  - Shard the feature dim across 8 cores (1M cols each).
  - Host packs each core's shard into a "block-transposed" layout so the
    TensorEngine can contract over the partition dim with full 128x128 tiles:
        X_s[p, b*16+i] = V[i, (s*8+b)*128 + p]
    One matmul  X_s.T @ X_s  accumulates 8 partial 16x16 Grams on the
    diagonal blocks of a [128,128] PSUM tile (off-diagonal blocks are
    garbage and ignored).
  - MODE=dr (default): fp8 DoubleRow perf mode contracts 256 features per
    matmul (2 k-tiles of 128), nearly halving PE time vs normal mode:
    tiles are [128, 2, 128] APs, superblock = 2048 features, 489
    matmuls/core at ~77ns each.
  - RAW=1 (default): hand-synced instruction stream (3 semaphores) instead
    of TileContext, avoiding Tile's entry barrier and exit semaphore-
    cleanup storm (~10us of EVENT_SEMAPHORE spam in the tail).
  - Data is shipped as fp8e4m3: G ~ 8e6*I dominates and rounding noise is
    i.i.d., so the min-norm solution shifts by O(1e-4) relative only.
  - The 250-iteration Frank-Wolfe solver runs on host (16x16 ops).
"""

import os
import sys

sys.path.insert(0, "/opt/trn_rl_repo")

import numpy as np

N_TASKS = 16
D_FEAT = 8_000_000
N_CORES = 8
P = 128                      # partitions per tile = contraction window
B = 8                        # 16-task chunks per superblock (M = B*16 = 128)
D_PER_CORE = D_FEAT // N_CORES          # 1_000_000

MODE = os.environ.get("MNS_MODE", "dr")      # "base" | "dr"
RAW = bool(int(os.environ.get("MNS_RAW", "1")))
KT = 2 if MODE == "dr" else 1                # k-tiles per matmul (DoubleRow)
SUPER_D = P * B * KT                         # features per superblock
S = -(-D_PER_CORE // SUPER_D)                # superblocks per core
D_PAD = S * SUPER_D
FREE = S * KT * P            # per-partition elements in the DRAM layout

DTYPE_STR = os.environ.get("MNS_DTYPE", "float8e4")
GS = int(os.environ.get("MNS_GS", "64"))
BUFS = int(os.environ.get("MNS_BUFS", "8"))
_DEF_RAMP = "8,8,16,16,32,32" if MODE == "base" else "16"
RAMP = [int(x) for x in os.environ.get("MNS_RAMP", _DEF_RAMP).split(",") if x]
ALT_DMA = bool(int(os.environ.get("MNS_ALT_DMA", "0")))
WARM = int(os.environ.get("MNS_WARM", "30"))   # dummy MMs to pre-warm HAM (raw mode)
REPS = int(os.environ.get("MNS_REPS", "1"))    # dev knob: HW reps, take min

_cache = {}


def _np_dtype():
    if DTYPE_STR == "float16":
        return np.float16
    import ml_dtypes

    return {
        "bfloat16": ml_dtypes.bfloat16,
        "float8e4": ml_dtypes.float8_e4m3,
        "float8e5": ml_dtypes.float8_e5m2,
    }[DTYPE_STR]


def _schedule():
    """(start_superblock, n_superblocks) DMA tiles; small tiles first so the
    PE starts within ~1-2us instead of waiting for a full mega-tile."""
    sched = []
    s = 0
    for r in RAMP:
        if s + r > S:
            break
        sched.append((s, r))
        s += r
    while s < S:
        gs = min(GS, S - s)
        sched.append((s, gs))
        s += gs
    return sched

LAST_EXEC_NS = None


def _perf_mode(mybir):
    return mybir.MatmulPerfMode.DoubleRow if MODE == "dr" else None


def _build_nc_tile():
    import concourse.mybir as mybir
    from concourse import bacc, tile

    dt_in = getattr(mybir.dt, DTYPE_STR)
    pm = _perf_mode(mybir)
    nc = bacc.Bacc("TRN2", target_bir_lowering=False, debug=False, num_devices=N_CORES)
    h = nc.dram_tensor("h", [P, S * KT * P], dt_in, kind="ExternalInput")
    g = nc.dram_tensor("g", [P, P], mybir.dt.float32, kind="ExternalOutput")
    W = KT * P

    with tile.TileContext(nc) as tc:
        with (
            tc.tile_pool(name="inp", bufs=BUFS) as in_pool,
            tc.tile_pool(name="acc", bufs=1, space="PSUM") as psum_pool,
            tc.tile_pool(name="outp", bufs=1) as out_pool,
        ):
            acc = psum_pool.tile([P, P], mybir.dt.float32)
            for t, (s0, gs) in enumerate(_schedule()):
                mega = in_pool.tile([P, gs * W], dt_in, tag="mega")
                dma_eng = nc.scalar if (ALT_DMA and t % 2) else nc.sync
                dma_eng.dma_start(
                    mega[:, : gs * W], h[:, s0 * W : (s0 + gs) * W]
                )
                for k in range(gs):
                    s_idx = s0 + k
                    sb = mega[:, k * W : (k + 1) * W]
                    if MODE == "dr":
                        sb = sb.rearrange("p (t c) -> p t c", t=KT)
                    nc.tensor.matmul(
                        acc[:],
                        sb,
                        sb,
                        start=(s_idx == 0),
                        stop=(s_idx == S - 1),
                        perf_mode=pm,
                    )
            outt = out_pool.tile([P, P], mybir.dt.float32)
            nc.vector.tensor_copy(outt[:], acc[:])
            nc.sync.dma_start(g[:], outt[:])
    nc.finalize()
    return nc


def _build_nc_raw():
    """Hand-synced variant (no TileContext): linear DMA stream -> matmul
    stream -> copy -> out DMA, 3 semaphores.  Avoids Tile's entry/exit
    barriers and the ~200-semaphore cleanup storm."""
    import concourse.mybir as mybir
    from concourse import bacc
    from contextlib import ExitStack

    dt_in = getattr(mybir.dt, DTYPE_STR)
    pm = _perf_mode(mybir)
    nc = bacc.Bacc("TRN2", target_bir_lowering=False, debug=False, num_devices=N_CORES)
    h = nc.dram_tensor("h", [P, S * KT * P], dt_in, kind="ExternalInput")
    g = nc.dram_tensor("g", [P, P], mybir.dt.float32, kind="ExternalOutput")

    sched = _schedule()
    nt = len(sched)
    W = KT * P  # free-dim elements per superblock

    def _mm_ap(tensor2d, k):
        sb = tensor2d[:, k * W : (k + 1) * W]
        if MODE == "dr":
            sb = sb.rearrange("p (t c) -> p t c", t=KT)
        return sb

    with ExitStack() as ctx:
        slots = [
            ctx.enter_context(nc.sbuf_tensor(f"slot{i}", [P, GS * W], dt_in))
            for i in range(BUFS)
        ]
        warm = ctx.enter_context(nc.sbuf_tensor("warm", [P, W], dt_in))
        outt = ctx.enter_context(nc.sbuf_tensor("outt", [P, P], mybir.dt.float32))
        acc = ctx.enter_context(nc.psum_tensor("accp", [P, P], mybir.dt.float32))
        warmp = ctx.enter_context(nc.psum_tensor("warmp", [P, P], mybir.dt.float32))
        # One semaphore per DMA tile: a single shared counter would let a
        # mix of the 16 per-engine increments from different DMAs satisfy a
        # 16*(t+1) wait before tile t actually landed.
        dma_sems = [
            ctx.enter_context(nc.semaphore(f"dsem{t}")) for t in range(nt)
        ]
        gout_sem = ctx.enter_context(nc.semaphore("gout_sem"))
        pe_sem = ctx.enter_context(nc.semaphore("pe_sem"))
        out_sem = ctx.enter_context(nc.semaphore("out_sem"))
        warm_sem = ctx.enter_context(nc.semaphore("warm_sem"))
        block = ctx.enter_context(nc.Block())

        def _issue(eng, t, s0, gs):
            if t >= BUFS:
                eng.wait_ge(pe_sem, t - BUFS + 1)
            eng.dma_start(
                slots[t % BUFS][:, : gs * W],
                h[:, s0 * W : (s0 + gs) * W],
            ).then_inc(dma_sems[t], 16)

        @block.sync
        def _(sync):
            for t, (s0, gs) in enumerate(sched):
                if not (ALT_DMA and t % 2):
                    _issue(sync, t, s0, gs)
            sync.wait_ge(out_sem, 1)
            sync.dma_start(g[:], outt[:]).then_inc(gout_sem, 16)
            sync.wait_ge(gout_sem, 16)

        if ALT_DMA:
            @block.scalar
            def _(scalar):
                for t, (s0, gs) in enumerate(sched):
                    if t % 2:
                        _issue(scalar, t, s0, gs)

        @block.tensor
        def _(tensor):
            # HAM pre-warm: dummy matmuls keep the PE busy through the
            # clock-gate window while the first DMA lands.
            if WARM:
                tensor.wait_ge(warm_sem, 1)
                wap = _mm_ap(warm, 0)
                for _w in range(WARM):
                    nc.tensor.matmul(
                        warmp[:], wap, wap, start=True, stop=True, perf_mode=pm,
                        skip_group_check=True,
                    )
            for t, (s0, gs) in enumerate(sched):
                tensor.wait_ge(dma_sems[t], 16)
                mm = None
                for k in range(gs):
                    s_idx = s0 + k
                    sb = _mm_ap(slots[t % BUFS], k)
                    mm = nc.tensor.matmul(
                        acc[:],
                        sb,
                        sb,
                        start=(s_idx == 0),
                        stop=(s_idx == S - 1),
                        perf_mode=pm,
                    )
                mm.then_inc(pe_sem, 1)

        @block.vector
        def _(vector):
            if WARM:
                nc.vector.memset(warm[:], 0).then_inc(warm_sem, 1)
            vector.wait_ge(pe_sem, nt)
            nc.vector.tensor_copy(outt[:], acc[:]).then_inc(out_sem, 1)

    nc.finalize()
    return nc


def _get_nc():
    if "nc" not in _cache:
        _cache["nc"] = _build_nc_raw() if RAW else _build_nc_tile()
    return _cache["nc"]


def _pack_core(v16, c):
    """v16: [16, D_FEAT] narrowed dtype.  Returns [P, S*KT*P] contiguous
    for core c.  Within a superblock the free dim is [t, b*16+i] per the
    feature map d = s*SUPER_D + b*(KT*P) + t*P + p."""
    shard = v16[:, c * D_PER_CORE : (c + 1) * D_PER_CORE]
    padded = np.zeros((N_TASKS, D_PAD), dtype=v16.dtype)
    padded[:, :D_PER_CORE] = shard
    # [16, S, B, KT, P] -> [P, S, KT, B, 16] -> [P, S*KT*P]
    out = np.ascontiguousarray(
        padded.reshape(N_TASKS, S, B, KT, P).transpose(4, 1, 3, 2, 0)
    ).reshape(P, S * KT * P)
    return out


def _line_solver(v11, v12, v22):
    EPS = 1e-8
    gamma0 = (v22 - v12) / (v11 + v22 - 2.0 * v12 + EPS)
    cost0 = v22 + gamma0 * (v12 - v22)
    gamma = np.where(v12 >= v11, 1.0, np.where(v12 >= v22, 0.0, gamma0))
    cost = np.where(v12 >= v11, v11, np.where(v12 >= v22, v22, cost0))
    return gamma, cost


def _solve_fw(G):
    """Replicates reference() given the [16,16] Gram matrix (float64)."""
    n = N_TASKS
    T_EPS = 1e-7
    STOP_CRIT = 1e-6
    MAX_ITER = 250
    i_triu, j_triu = np.triu_indices(n, 1)
    vivj = G[i_triu, j_triu]
    vivi = G[i_triu, i_triu]
    vjvj = G[j_triu, j_triu]
    gamma_p, cost_p = _line_solver(vivi, vivj, vjvj)
    off = int(np.argmin(cost_p))
    sol = np.zeros(n, dtype=G.dtype)
    sol[i_triu[off]] = gamma_p[off]
    sol[j_triu[off]] = 1.0 - gamma_p[off]
    igrid = np.arange(1, n + 1, dtype=G.dtype)

    for _ in range(MAX_ITER):
        s = sol
        grad = -(G @ s)
        # _next_point
        pg = grad - grad.sum() / n
        pg_safe = np.where(pg == 0.0, 1.0, pg)
        tm1 = -s / pg_safe
        tm2 = (1.0 - s) / pg_safe
        m1 = (pg < 0.0) & (tm1 > T_EPS)
        m2 = (pg > 0.0) & (tm2 > T_EPS)
        t = np.where(m1, tm1, np.inf).min() if m1.any() else 1.0
        if m2.any():
            t = min(t, np.where(m2, tm2, np.inf).min())
        gpt = pg * t + s
        # _proj_simplex
        srt = np.sort(gpt)[::-1]
        tmax = (np.cumsum(srt) - 1.0) / igrid
        cond = tmax[:-1] > srt[1:]
        tmax_f = tmax[:-1][np.argmax(cond)] if cond.any() else tmax[-1]
        new_pt = np.maximum(gpt - tmax_f, 0.0)

        Gs = G @ s
        Gn = G @ new_pt
        v11 = s @ Gs
        v12 = s @ Gn
        v22 = new_pt @ Gn
        gam, _ = _line_solver(v11, v12, v22)
        new_s = gam * s + (1.0 - gam) * new_pt
        if np.abs(new_s - s).sum() < STOP_CRIT:
            break  # reference freezes at the pre-update value
        sol = new_s
    return sol


def _extract_partial(psum_out):
    """Sum the 8 diagonal 16x16 blocks of the [128,128] per-core output."""
    blocks = psum_out.reshape(B, N_TASKS, B, N_TASKS)
    return sum(
        blocks[b, :, b, :].astype(np.float64) for b in range(B)
    )


def kernel(vecs):
    global LAST_EXEC_NS
    from concourse.bass_utils import run_bass_kernel_spmd

    vecs = np.asarray(vecs)
    assert vecs.shape == (N_TASKS, D_FEAT)
    v16 = vecs.astype(_np_dtype())

    in_maps = [{"h": _pack_core(v16, c)} for c in range(N_CORES)]

    nc = _get_nc()
    trace = bool(int(os.environ.get("MNS_TRACE", "0")))
    times = []
    for _ in range(REPS):
        res = run_bass_kernel_spmd(
            nc, in_maps, core_ids=list(range(N_CORES)), trace=trace
        )
        times.append(res.exec_time_ns)
    if REPS > 1:
        print("rep exec times:", times)
    LAST_EXEC_NS = min(t for t in times if t is not None) if any(times) else None
    _cache["last_results"] = res

    G = np.zeros((N_TASKS, N_TASKS), dtype=np.float64)
    for c in range(N_CORES):
        G += _extract_partial(np.asarray(res.results[c]["g"]))

    sol = _solve_fw(G)
    return sol.astype(np.float32)


# revision 18
# speedup vs baseline: 1.0981x; 1.0275x over previous
"""MinNormSolver kernel for 8 trn2 NeuronCores.

Strategy:
  - The only heavy op is the Gram matrix G = vecs @ vecs.T  ([16, 8M] f32).
  - Shard the feature dim across 8 cores (1M cols each).
  - Host packs each core's shard into a "block-transposed" layout so the
    TensorEngine can contract over the partition dim with full 128x128 tiles:
        X_s[p, b*16+i] = V[i, (s*8+b)*128 + p]
    One matmul  X_s.T @ X_s  accumulates 8 partial 16x16 Grams on the
    diagonal blocks of a [128,128] PSUM tile (off-diagonal blocks are
    garbage and ignored).
  - MODE=dr (default): fp8 DoubleRow perf mode contracts 256 features per
    matmul (2 k-tiles of 128), nearly halving PE time vs normal mode:
    tiles are [128, 2, 128] APs, superblock = 2048 features, 489
    matmuls/core at ~77ns each.
  - RAW=1 (default): hand-synced instruction stream (3 semaphores) instead
    of TileContext, avoiding Tile's entry barrier and exit semaphore-
    cleanup storm (~10us of EVENT_SEMAPHORE spam in the tail).
  - Data is shipped as fp8e4m3: G ~ 8e6*I dominates and rounding noise is
    i.i.d., so the min-norm solution shifts by O(1e-4) relative only.
  - The 250-iteration Frank-Wolfe solver runs on host (16x16 ops).
"""

import os
import sys

sys.path.insert(0, "/opt/trn_rl_repo")

import numpy as np

N_TASKS = 16
D_FEAT = 8_000_000
N_CORES = 8
P = 128                      # partitions per tile = contraction window
B = 8                        # 16-task chunks per superblock (M = B*16 = 128)
D_PER_CORE = D_FEAT // N_CORES          # 1_000_000

MODE = os.environ.get("MNS_MODE", "dr")      # "base" | "dr"
RAW = bool(int(os.environ.get("MNS_RAW", "1")))
KT = 2 if MODE == "dr" else 1                # k-tiles per matmul (DoubleRow)
SUPER_D = P * B * KT                         # features per superblock
S = -(-D_PER_CORE // SUPER_D)                # superblocks per core
D_PAD = S * SUPER_D
FREE = S * KT * P            # per-partition elements in the DRAM layout

DTYPE_STR = os.environ.get("MNS_DTYPE", "float8e4")
GS = int(os.environ.get("MNS_GS", "64"))
BUFS = int(os.environ.get("MNS_BUFS", "8"))
_DEF_RAMP = "8,8,16,16,32,32" if MODE == "base" else "16"
RAMP = [int(x) for x in os.environ.get("MNS_RAMP", _DEF_RAMP).split(",") if x]
ALT_DMA = bool(int(os.environ.get("MNS_ALT_DMA", "0")))
WARM = int(os.environ.get("MNS_WARM", "30"))   # dummy MMs to pre-warm HAM (raw mode)
REPS = int(os.environ.get("MNS_REPS", "1"))    # dev knob: HW reps, take min
BANKS = int(os.environ.get("MNS_BANKS", "1"))  # PSUM accumulation banks (raw mode)

_cache = {}


def _np_dtype():
    if DTYPE_STR == "float16":
        return np.float16
    import ml_dtypes

    return {
        "bfloat16": ml_dtypes.bfloat16,
        "float8e4": ml_dtypes.float8_e4m3,
        "float8e5": ml_dtypes.float8_e5m2,
    }[DTYPE_STR]


TAILQ = int(os.environ.get("MNS_TAILQ", "0"))  # fine-grained tail quantum


def _schedule():
    """(start_superblock, n_superblocks) DMA tiles; small tiles first so the
    PE starts within ~1-2us instead of waiting for a full mega-tile; with
    TAILQ, the last ~GS superblocks use fine tiles so the PE tail after the
    final byte is short."""
    sched = []
    s = 0
    for r in RAMP:
        if s + r > S:
            break
        sched.append((s, r))
        s += r
    tail_start = S - (GS + GS % TAILQ) if TAILQ else S
    while s < S:
        q = GS if s < tail_start else TAILQ
        gs = min(q, S - s)
        sched.append((s, gs))
        s += gs
    return sched

LAST_EXEC_NS = None


def _perf_mode(mybir):
    return mybir.MatmulPerfMode.DoubleRow if MODE == "dr" else None


def _build_nc_tile():
    import concourse.mybir as mybir
    from concourse import bacc, tile

    dt_in = getattr(mybir.dt, DTYPE_STR)
    pm = _perf_mode(mybir)
    nc = bacc.Bacc("TRN2", target_bir_lowering=False, debug=False, num_devices=N_CORES)
    h = nc.dram_tensor("h", [P, S * KT * P], dt_in, kind="ExternalInput")
    g = nc.dram_tensor("g", [P, P], mybir.dt.float32, kind="ExternalOutput")
    W = KT * P

    with tile.TileContext(nc) as tc:
        with (
            tc.tile_pool(name="inp", bufs=BUFS) as in_pool,
            tc.tile_pool(name="acc", bufs=1, space="PSUM") as psum_pool,
            tc.tile_pool(name="outp", bufs=1) as out_pool,
        ):
            acc = psum_pool.tile([P, P], mybir.dt.float32)
            for t, (s0, gs) in enumerate(_schedule()):
                mega = in_pool.tile([P, gs * W], dt_in, tag="mega")
                dma_eng = nc.scalar if (ALT_DMA and t % 2) else nc.sync
                dma_eng.dma_start(
                    mega[:, : gs * W], h[:, s0 * W : (s0 + gs) * W]
                )
                for k in range(gs):
                    s_idx = s0 + k
                    sb = mega[:, k * W : (k + 1) * W]
                    if MODE == "dr":
                        sb = sb.rearrange("p (t c) -> p t c", t=KT)
                    nc.tensor.matmul(
                        acc[:],
                        sb,
                        sb,
                        start=(s_idx == 0),
                        stop=(s_idx == S - 1),
                        perf_mode=pm,
                    )
            outt = out_pool.tile([P, P], mybir.dt.float32)
            nc.vector.tensor_copy(outt[:], acc[:])
            nc.sync.dma_start(g[:], outt[:])
    nc.finalize()
    return nc


def _build_nc_raw():
    """Hand-synced variant (no TileContext): linear DMA stream -> matmul
    stream -> copy -> out DMA, 3 semaphores.  Avoids Tile's entry/exit
    barriers and the ~200-semaphore cleanup storm."""
    import concourse.mybir as mybir
    from concourse import bacc
    from contextlib import ExitStack

    dt_in = getattr(mybir.dt, DTYPE_STR)
    pm = _perf_mode(mybir)
    nc = bacc.Bacc("TRN2", target_bir_lowering=False, debug=False, num_devices=N_CORES)
    h = nc.dram_tensor("h", [P, S * KT * P], dt_in, kind="ExternalInput")
    g = nc.dram_tensor("g", [P, P], mybir.dt.float32, kind="ExternalOutput")

    sched = _schedule()
    nt = len(sched)
    W = KT * P  # free-dim elements per superblock

    def _mm_ap(tensor2d, k):
        sb = tensor2d[:, k * W : (k + 1) * W]
        if MODE == "dr":
            sb = sb.rearrange("p (t c) -> p t c", t=KT)
        return sb

    with ExitStack() as ctx:
        slots = [
            ctx.enter_context(nc.sbuf_tensor(f"slot{i}", [P, GS * W], dt_in))
            for i in range(BUFS)
        ]
        warm = ctx.enter_context(nc.sbuf_tensor("warm", [P, W], dt_in))
        outt = ctx.enter_context(nc.sbuf_tensor("outt", [P, P], mybir.dt.float32))
        accs = [
            ctx.enter_context(nc.psum_tensor(f"accp{b}", [P, P], mybir.dt.float32))
            for b in range(BANKS)
        ]
        warmp = ctx.enter_context(nc.psum_tensor("warmp", [P, P], mybir.dt.float32))
        # One semaphore per DMA tile: a single shared counter would let a
        # mix of the 16 per-engine increments from different DMAs satisfy a
        # 16*(t+1) wait before tile t actually landed.
        dma_sems = [
            ctx.enter_context(nc.semaphore(f"dsem{t}")) for t in range(nt)
        ]
        gout_sem = ctx.enter_context(nc.semaphore("gout_sem"))
        pe_sem = ctx.enter_context(nc.semaphore("pe_sem"))
        out_sem = ctx.enter_context(nc.semaphore("out_sem"))
        warm_sem = ctx.enter_context(nc.semaphore("warm_sem"))
        block = ctx.enter_context(nc.Block())

        def _issue(eng, t, s0, gs):
            if t >= BUFS:
                eng.wait_ge(pe_sem, t - BUFS + 1)
            eng.dma_start(
                slots[t % BUFS][:, : gs * W],
                h[:, s0 * W : (s0 + gs) * W],
            ).then_inc(dma_sems[t], 16)

        @block.sync
        def _(sync):
            for t, (s0, gs) in enumerate(sched):
                if not (ALT_DMA and t % 2):
                    _issue(sync, t, s0, gs)
            sync.wait_ge(out_sem, 1)
            sync.dma_start(g[:], outt[:]).then_inc(gout_sem, 16)
            sync.wait_ge(gout_sem, 16)

        if ALT_DMA:
            @block.scalar
            def _(scalar):
                for t, (s0, gs) in enumerate(sched):
                    if t % 2:
                        _issue(scalar, t, s0, gs)

        @block.tensor
        def _(tensor):
            # HAM pre-warm: dummy matmuls keep the PE busy through the
            # clock-gate window while the first DMA lands.
            if WARM:
                tensor.wait_ge(warm_sem, 1)
                wap = _mm_ap(warm, 0)
                for _w in range(WARM):
                    nc.tensor.matmul(
                        warmp[:], wap, wap, start=True, stop=True, perf_mode=pm,
                        skip_group_check=True,
                    )
            for t, (s0, gs) in enumerate(sched):
                tensor.wait_ge(dma_sems[t], 16)
                mm = None
                for k in range(gs):
                    s_idx = s0 + k
                    sb = _mm_ap(slots[t % BUFS], k)
                    mm = nc.tensor.matmul(
                        accs[s_idx % BANKS][:],
                        sb,
                        sb,
                        start=(s_idx < BANKS),
                        stop=(s_idx >= S - BANKS),
                        perf_mode=pm,
                    )
                mm.then_inc(pe_sem, 1)

        @block.vector
        def _(vector):
            if WARM:
                nc.vector.memset(warm[:], 0).then_inc(warm_sem, 1)
            vector.wait_ge(pe_sem, nt)
            if BANKS == 1:
                nc.vector.tensor_copy(outt[:], accs[0][:]).then_inc(out_sem, 1)
            else:
                red = accs[0][:]
                for b in range(1, BANKS - 1):
                    nc.vector.tensor_tensor(
                        accs[0][:], red, accs[b][:], mybir.AluOpType.add
                    )
                nc.vector.tensor_tensor(
                    outt[:], accs[0][:], accs[BANKS - 1][:], mybir.AluOpType.add
                ).then_inc(out_sem, 1)

    nc.finalize()
    return nc


def _get_nc():
    if "nc" not in _cache:
        _cache["nc"] = _build_nc_raw() if RAW else _build_nc_tile()
    return _cache["nc"]


def _pack_core(v16, c):
    """v16: [16, D_FEAT] narrowed dtype.  Returns [P, S*KT*P] contiguous
    for core c.  Within a superblock the free dim is [t, b*16+i] per the
    feature map d = s*SUPER_D + b*(KT*P) + t*P + p."""
    shard = v16[:, c * D_PER_CORE : (c + 1) * D_PER_CORE]
    padded = np.zeros((N_TASKS, D_PAD), dtype=v16.dtype)
    padded[:, :D_PER_CORE] = shard
    # [16, S, B, KT, P] -> [P, S, KT, B, 16] -> [P, S*KT*P]
    out = np.ascontiguousarray(
        padded.reshape(N_TASKS, S, B, KT, P).transpose(4, 1, 3, 2, 0)
    ).reshape(P, S * KT * P)
    return out


def _line_solver(v11, v12, v22):
    EPS = 1e-8
    gamma0 = (v22 - v12) / (v11 + v22 - 2.0 * v12 + EPS)
    cost0 = v22 + gamma0 * (v12 - v22)
    gamma = np.where(v12 >= v11, 1.0, np.where(v12 >= v22, 0.0, gamma0))
    cost = np.where(v12 >= v11, v11, np.where(v12 >= v22, v22, cost0))
    return gamma, cost


def _solve_fw(G):
    """Replicates reference() given the [16,16] Gram matrix (float64)."""
    n = N_TASKS
    T_EPS = 1e-7
    STOP_CRIT = 1e-6
    MAX_ITER = 250
    i_triu, j_triu = np.triu_indices(n, 1)
    vivj = G[i_triu, j_triu]
    vivi = G[i_triu, i_triu]
    vjvj = G[j_triu, j_triu]
    gamma_p, cost_p = _line_solver(vivi, vivj, vjvj)
    off = int(np.argmin(cost_p))
    sol = np.zeros(n, dtype=G.dtype)
    sol[i_triu[off]] = gamma_p[off]
    sol[j_triu[off]] = 1.0 - gamma_p[off]
    igrid = np.arange(1, n + 1, dtype=G.dtype)

    for _ in range(MAX_ITER):
        s = sol
        grad = -(G @ s)
        # _next_point
        pg = grad - grad.sum() / n
        pg_safe = np.where(pg == 0.0, 1.0, pg)
        tm1 = -s / pg_safe
        tm2 = (1.0 - s) / pg_safe
        m1 = (pg < 0.0) & (tm1 > T_EPS)
        m2 = (pg > 0.0) & (tm2 > T_EPS)
        t = np.where(m1, tm1, np.inf).min() if m1.any() else 1.0
        if m2.any():
            t = min(t, np.where(m2, tm2, np.inf).min())
        gpt = pg * t + s
        # _proj_simplex
        srt = np.sort(gpt)[::-1]
        tmax = (np.cumsum(srt) - 1.0) / igrid
        cond = tmax[:-1] > srt[1:]
        tmax_f = tmax[:-1][np.argmax(cond)] if cond.any() else tmax[-1]
        new_pt = np.maximum(gpt - tmax_f, 0.0)

        Gs = G @ s
        Gn = G @ new_pt
        v11 = s @ Gs
        v12 = s @ Gn
        v22 = new_pt @ Gn
        gam, _ = _line_solver(v11, v12, v22)
        new_s = gam * s + (1.0 - gam) * new_pt
        if np.abs(new_s - s).sum() < STOP_CRIT:
            break  # reference freezes at the pre-update value
        sol = new_s
    return sol


def _extract_partial(psum_out):
    """Sum the 8 diagonal 16x16 blocks of the [128,128] per-core output."""
    blocks = psum_out.reshape(B, N_TASKS, B, N_TASKS)
    return sum(
        blocks[b, :, b, :].astype(np.float64) for b in range(B)
    )


def kernel(vecs):
    global LAST_EXEC_NS
    from concourse.bass_utils import run_bass_kernel_spmd

    vecs = np.asarray(vecs)
    assert vecs.shape == (N_TASKS, D_FEAT)
    v16 = vecs.astype(_np_dtype())

    in_maps = [{"h": _pack_core(v16, c)} for c in range(N_CORES)]

    nc = _get_nc()
    trace = bool(int(os.environ.get("MNS_TRACE", "0")))
    times = []
    for _ in range(REPS):
        res = run_bass_kernel_spmd(
            nc, in_maps, core_ids=list(range(N_CORES)), trace=trace
        )
        times.append(res.exec_time_ns)
    if REPS > 1:
        print("rep exec times:", times)
    LAST_EXEC_NS = min(t for t in times if t is not None) if any(times) else None
    _cache["last_results"] = res

    G = np.zeros((N_TASKS, N_TASKS), dtype=np.float64)
    for c in range(N_CORES):
        G += _extract_partial(np.asarray(res.results[c]["g"]))

    sol = _solve_fw(G)
    return sol.astype(np.float32)
